# revision 18
# baseline (speedup 1.0000x reference)
"""Trainium2 Bass kernel for the KB criterion loss.

Math
----
reference:
    diff[b,i,j] = probs[b,j] - probs[b,i]
    loss = sum_ij mean_b (diff^2 * C[i,j]) / (n_pos + 1e-8),  n_pos = count(C > 0)

Expanding the square removes the [B,N,N] intermediate entirely:

    sum_b (P[b,i] - P[b,j])^2 = S2_i + S2_j - 2*G_ij
        with S2_j = sum_b P[b,j]^2   and   G = P^T P  (Gram matrix)

so   total = sum_ij C_ij * D_ij,   D = S2_i + S2_j - 2 G_ij
     loss  = (total / B) / (n_pos + 1e-8)

Sharding (8 cores)
------------------
Shard C by rows: core k owns rows S_k = [128k, 128k+128). P is replicated.
Inputs are column-rolled by 128k so every core runs the same program with
its own row block mapped to local columns [0:128).

v2 (vs the 30.5us f32 baseline, trace-driven):
  * bf16 inputs (host downcast): halves DMA bytes AND turns the 16
    multi-pass fp32 HW matmuls (~12us cold) into 8 bf16 matmuls.
  * PE warmup: dummy matmuls on a zeroed tile during the DMA wait so the
    HAM clock gate reaches 8/8 (2.4 GHz) before the real matmuls issue.
  * tensor_tensor_reduce fuses (C * D') * -2 with the per-partition
    reduction in one DVE op per PSUM bank half; half 0 reduces while the
    PE still accumulates half 1.
  * P is DMA'd in two halves so squaring/S2 start ~0.35us earlier.

Per-core pipeline:
  1. DMA P half0, P half1, C (bf16).
  2. DVE: Psq = P*P per half.
  3. PE:  S2h = -(1/2) * ones^T @ Psq per half -> [1, N] PSUM.
  4. ACT: copy S2h PSUM->SBUF (bf16) per half.
  5. PE:  D' per 512-col bank: Gram + s2h_i x 1 + 1 x s2h_j  (= -D/2).
  6. DVE: tensor_tensor_reduce: (C * D') * -2, accum -> partials col h.
  7. ACT: Sign(C) accum -> partials col 2 (n_pos).
  8. PE:  ones^T reductions -> [1,2] + [1,1]; ACT copies; DMA [1,3] out.

Host sums the 8 partial triples (the scalar all-reduce) and finishes the
division.
"""

import numpy as np
import ml_dtypes

import concourse.bass as bass
import concourse.tile as tile
from concourse import mybir
from concourse.bass_utils import run_bass_kernel_spmd

B = 128
N = 1024
NCORES = 8
SH = N // NCORES  # 128 rows of C per core
F32 = mybir.dt.float32
BF16 = mybir.dt.bfloat16
HALF = 512  # PSUM bank width in fp32
WARMUP = 6  # dummy matmuls that warm the PE clock gate during DMA wait


def build_bass() -> bass.Bass:
    nc = bass.Bass()
    p_d = nc.dram_tensor("probs_r", [B, N], BF16, kind="ExternalInput")
    c_d = nc.dram_tensor("co_r", [SH, N], BF16, kind="ExternalInput")
    o_d = nc.dram_tensor("out", [B, 3], F32, kind="ExternalOutput")

    with tile.TileContext(nc) as tc:
        with (
            tc.tile_pool(name="sb", bufs=1) as sb,
            tc.tile_pool(name="ps", bufs=1, space="PSUM") as ps,
        ):
            p_sb = sb.tile([B, N], BF16)
            c_sb = sb.tile([SH, N], BF16)
            psq = sb.tile([B, N], BF16)
            s2h = sb.tile([1, N], BF16)
            ones_row = sb.tile([1, HALF], BF16)
            nh_col = sb.tile([B, 1], BF16)
            scr_mul = sb.tile([SH, N], BF16)
            scr_cnt = sb.tile([SH, N], BF16)
            scr_red = sb.tile([SH, N], BF16)
            partials = sb.tile([B, 4], F32)

            d_ps = ps.tile([B, N], F32)  # banks 0-1
            s2_ps0 = ps.tile([1, HALF], F32)
            s2_ps1 = ps.tile([1, HALF], F32)

            # Constants. trn2 LDWEIGHTS carries ONE sync-wait slot, so each
            # matmul's operands should trace to a single upstream engine:
            # DVE-born consts pair with DVE-produced psq.
            # (No PE warmup: the HAM clock gate measurably never opens for
            # this kernel — bf16 dummy matmuls over 5us of contiguous busy
            # produced zero HAM transitions — so dummies only delayed S2.)
            nc.vector.memset(nh_col, -0.5)
            nc.vector.memset(ones_row, 1.0)

            # Loads: P halves first (they head the critical path), then C.
            nc.sync.dma_start(out=p_sb[:, 0:HALF], in_=p_d[:, 0:HALF])
            nc.sync.dma_start(out=p_sb[:, HALF:N], in_=p_d[:, HALF:N])
            nc.sync.dma_start(out=c_sb, in_=c_d[:, :])

            # Psq = P*P per half
            nc.vector.tensor_mul(psq[:, 0:HALF], p_sb[:, 0:HALF], p_sb[:, 0:HALF])
            nc.vector.tensor_mul(psq[:, HALF:N], p_sb[:, HALF:N], p_sb[:, HALF:N])

            # -S2/2 = (-1/2)*colsum_b(Psq) -> [1, N] PSUM, then to SBUF bf16.
            # The casts and the n_pos count run on the otherwise-idle ACT
            # engine: every DVE op with a PSUM operand or an accumulator
            # measured at 1x anyway, and DVE is the bottleneck engine.
            nc.tensor.matmul(s2_ps0, nh_col, psq[:, 0:HALF], start=True, stop=True)
            nc.tensor.matmul(s2_ps1, nh_col, psq[:, HALF:N], start=True, stop=True)
            nc.scalar.copy(s2h[0:1, 0:HALF], s2_ps0)
            nc.scalar.copy(s2h[0:1, HALF:N], s2_ps1)

            # n_pos per partition: sum_j sign(C)  (C >= 0 always)
            nc.scalar.activation(
                scr_cnt, c_sb, mybir.ActivationFunctionType.Sign,
                accum_out=partials[:, 2:3],
            )

            # D' = G - S2_i/2 - S2_j/2 (= -D/2) accumulated per PSUM bank.
            # The fused DVE op computes (-2*D')*C = C*D elementwise AND the
            # per-partition row sum; half 0 reduces while the PE still
            # accumulates half 1 into the other bank.
            for h in range(2):
                js = slice(HALF * h, HALF * (h + 1))
                nc.tensor.matmul(
                    d_ps[:, js], p_sb[:, 0:SH], p_sb[:, js], start=True, stop=False
                )
                nc.tensor.matmul(
                    d_ps[:, js], s2h[0:1, 0:SH], ones_row[0:1, :],
                    start=False, stop=False,
                )
                nc.tensor.matmul(
                    d_ps[:, js], ones_row[0:1, 0:SH], s2h[0:1, js],
                    start=False, stop=True,
                )
                nc.vector.tensor_mul(scr_mul[:, js], c_sb[:, js], d_ps[:, js])
                # cheap row-reduce: tensor_scalar mult-by-1 with accum runs
                # at 4x for bf16 SBUF (~194ns/half vs 720ns for an ACT pass)
                nc.vector.tensor_scalar(
                    scr_red[:, js], scr_mul[:, js], 1.0, None,
                    mybir.AluOpType.mult, mybir.AluOpType.add,
                    accum_out=partials[:, h : h + 1],
                )

            # ship the [128,3] per-partition partials; the host does the
            # final partition/core reduction (cheaper than a PE reduce +
            # PSUM->SBUF copy on the critical path)
            nc.sync.dma_start(out=o_d[:, :], in_=partials[:, 0:3])

    _split_multi_waits(nc)
    return nc


def _split_multi_waits(nc: bass.Bass):
    """This walrus build accepts only ONE sync-wait per instruction
    ("Too many sync wait commands"). Tile's kernel-tail drain carries one
    wait per live semaphore; peel the extras onto same-engine NOPs that
    each stall on a single semaphore — semantically identical."""
    for bb in nc.main_func.blocks:
        insts = bb.instructions
        i = 0
        while i < len(insts):
            ins = insts[i]
            si = getattr(ins, "sync_info", None)
            if si is not None and si.on_wait is not None and len(si.on_wait) > 1:
                waits = list(si.on_wait)
                nops = []
                for j, w in enumerate(waits[:-1]):
                    nop = mybir.InstNoOp(
                        name=f"{ins.name}-wsplit{j}",
                        sync_info=mybir.SyncInfo(on_wait=[w], on_update=[]),
                        bass_nofuse=True,
                        engine=ins.engine,
                    )
                    nc.register_instruction(nop, overwrite=True)
                    nops.append(nop)
                si.on_wait = [waits[-1]]
                insts[i:i] = nops
                i += len(nops)
            i += 1


_NC = None


def _get_nc() -> bass.Bass:
    global _NC
    if _NC is None:
        _NC = build_bass()
    return _NC


def make_in_maps(probs: np.ndarray, co_matrix: np.ndarray):
    probs = np.ascontiguousarray(np.asarray(probs, dtype=np.float32))
    co_matrix = np.ascontiguousarray(np.asarray(co_matrix, dtype=np.float32))
    in_maps = []
    for k in range(NCORES):
        shift = -SH * k
        p_r = np.ascontiguousarray(
            np.roll(probs, shift, axis=1).astype(ml_dtypes.bfloat16)
        )
        c_r = np.ascontiguousarray(
            np.roll(co_matrix[SH * k : SH * (k + 1), :], shift, axis=1).astype(
                ml_dtypes.bfloat16
            )
        )
        in_maps.append({"probs_r": p_r, "co_r": c_r})
    return in_maps


def finish(outs: np.ndarray) -> np.ndarray:
    """outs: [NCORES, 128, 3] per-partition (sum C*D' half0, half1, npos).

    D' = -D/2, so sum C*D = -2 * (col0 + col1)."""
    o = outs.astype(np.float64)
    total = np.float32(-2.0 * (o[:, :, 0] + o[:, :, 1]).sum())
    npos = np.float32(o[:, :, 2].sum())
    loss = (total / np.float32(B)) / (npos + np.float32(1e-8))
    return np.array(loss, dtype=np.float32)


TRACE = False
TRACE_DIR = None
LAST_RESULTS = None


def kernel(probs: np.ndarray, co_matrix: np.ndarray) -> np.ndarray:
    global LAST_RESULTS
    nc = _get_nc()
    in_maps = make_in_maps(probs, co_matrix)
    kwargs = {}
    if TRACE:
        kwargs = dict(trace=True, trace_cores=list(range(NCORES)), tmpdir=TRACE_DIR)
    res = run_bass_kernel_spmd(nc, in_maps, list(range(NCORES)), **kwargs)
    LAST_RESULTS = res
    outs = np.stack([r["out"] for r in res.results])
    return finish(outs)


# revision 22
# speedup vs baseline: 1.0429x; 1.0429x over previous
"""Trainium2 Bass kernel for the KB criterion loss.

Math
----
reference:
    diff[b,i,j] = probs[b,j] - probs[b,i]
    loss = sum_ij mean_b (diff^2 * C[i,j]) / (n_pos + 1e-8),  n_pos = count(C > 0)

Expanding the square removes the [B,N,N] intermediate entirely:

    sum_b (P[b,i] - P[b,j])^2 = T_i + T_j - 2*G_ij
        with T_j = sum_b P[b,j]^2   and   G = P^T P  (Gram matrix)

so   total = sum_ij C_ij*T_i + sum_ij C_ij*T_j - 2*sum_ij C_ij*G_ij
           =   A (rows)       +  Bt (cols)      -  2*CG
     loss  = (total / B) / (n_pos + 1e-8)

Sharding (8 cores)
------------------
Shard C by rows: core k owns rows S_k = [128k, 128k+128). P is replicated.
Inputs are column-rolled by 128k so every core runs the same program with
its own row block mapped to local columns [0:128) (so T for the shard rows
is just chunk 0 of the chunked T vector).

The three terms are computed with no cross-engine ladders (v6; each was
measured on HW traces):
  * CG: PE Gram per 512-col PSUM bank (2 bf16 matmuls), DVE multiply by C
    and tensor_scalar-accum row-reduce, pipelined per bank.
  * A = sum_i rowsum_i*T_i: ACT pass over C with the per-partition
    scale AP = (-T_shard/2) straight from PSUM, accum_out per row.
  * Bt = sum_j colsum_j*T_j: both vectors built in PARTITION orientation
    as [128,8] chunk columns via 16 tiny matmuls (contraction over the
    partition dim; chunk k lands in column k), then one tiny DVE
    multiply+reduce. This avoids any [1,N] row-vector op (a 1-partition
    DVE op costs the same as a 128-partition one).
  * n_pos: ACT Sign pass with accum_out.
The per-partition [128,6] partials go straight to HBM; the host does the
final partition/core reduction and the division (the sanctioned scalar
all-reduce).

bf16 inputs (host downcast): halves DMA bytes and avoids multi-pass fp32
matmuls. DMA order P-half0, C, P-half1: the C-gated DVE multiply is the
critical path, not P-half1 (only needed for Gram half 1 / T chunks 4-7).

No PE warmup: the HAM clock gate measurably never opens for this kernel
(bf16 dummy matmuls over 5us of contiguous busy produced zero HAM
transitions), so warmup matmuls only delayed the real work.
"""

import numpy as np
import ml_dtypes

import concourse.bass as bass
import concourse.tile as tile
from concourse import mybir
from concourse.bass_utils import run_bass_kernel_spmd

B = 128
N = 1024
NCORES = 8
SH = N // NCORES  # 128 rows of C per core
NCH = N // SH  # 8 column chunks
F32 = mybir.dt.float32
BF16 = mybir.dt.bfloat16
HALF = 512  # PSUM bank width in fp32


def build_bass() -> bass.Bass:
    nc = bass.Bass()
    p_d = nc.dram_tensor("probs_r", [B, N], BF16, kind="ExternalInput")
    c_d = nc.dram_tensor("co_r", [SH, N], BF16, kind="ExternalInput")
    o_d = nc.dram_tensor("out", [B, 6], F32, kind="ExternalOutput")

    with tile.TileContext(nc) as tc:
        with (
            tc.tile_pool(name="sb", bufs=1) as sb,
            tc.tile_pool(name="ps", bufs=1, space="PSUM") as ps,
        ):
            p_sb = sb.tile([B, N], BF16)
            c_sb = sb.tile([SH, N], BF16)
            psq = sb.tile([B, N], BF16)
            nh_col = sb.tile([B, 1], BF16)
            ones_col = sb.tile([B, 1], BF16)
            scr_mul = sb.tile([SH, N], BF16)
            scr_red = sb.tile([SH, N], BF16)
            scr_cnt = sb.tile([SH, N], BF16)
            scr_a = sb.tile([SH, N], BF16)
            th_sb = sb.tile([B, NCH], F32)
            scr_bt = sb.tile([B, NCH], F32)
            scr_bt2 = sb.tile([B, NCH], F32)
            partials = sb.tile([B, 6], F32)

            d_ps = ps.tile([B, N], F32)  # Gram, banks 0-1
            # th chunk 0 gets its own bank: the ACT A-term pass reads it as
            # a scale AP while the PE is still writing the other chunks —
            # separate banks keep that off the PSUM-collision/serialization
            # path.
            th0_ps = ps.tile([B, 1], F32)
            cols_ps = ps.tile([B, 2 * NCH - 1], F32)  # th 1-7, cs 0-7

            # Constants (DVE-born; matmul operands pair per upstream engine)
            nc.vector.memset(nh_col, -0.5)
            nc.vector.memset(ones_col, 1.0)

            # Loads. P half0 first (heads every chain), then C (gates the
            # critical DVE multiply), then P half1 (only Gram1/th4-7).
            nc.sync.dma_start(out=p_sb[:, 0:HALF], in_=p_d[:, 0:HALF])
            nc.sync.dma_start(out=c_sb, in_=c_d[:, :])
            nc.sync.dma_start(out=p_sb[:, HALF:N], in_=p_d[:, HALF:N])

            # Psq = P*P per half
            nc.vector.tensor_mul(psq[:, 0:HALF], p_sb[:, 0:HALF], p_sb[:, 0:HALF])
            nc.vector.tensor_mul(psq[:, HALF:N], p_sb[:, HALF:N], p_sb[:, HALF:N])

            # PE program. Gram halves head it; the 16 tiny chunk matmuls
            # build th = -T/2 and cs = colsum in partition orientation.
            nc.tensor.matmul(
                d_ps[:, 0:HALF], p_sb[:, 0:SH], p_sb[:, 0:HALF],
                start=True, stop=True,
            )
            nc.tensor.matmul(th0_ps, psq[:, 0:SH], nh_col, start=True, stop=True)
            for k in range(1, 4):
                nc.tensor.matmul(
                    cols_ps[:, k - 1 : k], psq[:, SH * k : SH * (k + 1)], nh_col,
                    start=True, stop=True,
                )
            nc.tensor.matmul(
                d_ps[:, HALF:N], p_sb[:, 0:SH], p_sb[:, HALF:N],
                start=True, stop=True,
            )
            for k in range(4, NCH):
                nc.tensor.matmul(
                    cols_ps[:, k - 1 : k], psq[:, SH * k : SH * (k + 1)], nh_col,
                    start=True, stop=True,
                )
            for k in range(NCH):
                nc.tensor.matmul(
                    cols_ps[:, NCH - 1 + k : NCH + k],
                    c_sb[:, SH * k : SH * (k + 1)], ones_col,
                    start=True, stop=True,
                )

            # DVE: stage -T_shard/2 to SBUF early — the ACT A-term scale AP
            # must be SBUF-resident.
            nc.vector.tensor_copy(th_sb[:, 0:1], th0_ps)

            # DVE: CG per bank (multiply then cheap accum-reduce)
            for h in range(2):
                js = slice(HALF * h, HALF * (h + 1))
                nc.vector.tensor_mul(scr_mul[:, js], c_sb[:, js], d_ps[:, js])
                nc.vector.tensor_scalar(
                    scr_red[:, js], scr_mul[:, js], 1.0, None,
                    mybir.AluOpType.mult, mybir.AluOpType.add,
                    accum_out=partials[:, h : h + 1],
                )

            # ACT: n_pos and the A-term (scale AP = -T_shard/2 from PSUM)
            nc.scalar.activation(
                scr_cnt, c_sb, mybir.ActivationFunctionType.Sign,
                accum_out=partials[:, 2:3],
            )
            nc.scalar.activation(
                scr_a, c_sb, mybir.ActivationFunctionType.Copy,
                scale=th_sb[:, 0:1], accum_out=partials[:, 3:4],
            )

            # DVE: Bt = sum over [128,8] of th * cs
            nc.vector.tensor_copy(th_sb[:, 1:NCH], cols_ps[:, 0 : NCH - 1])
            nc.vector.tensor_mul(
                scr_bt, th_sb, cols_ps[:, NCH - 1 : 2 * NCH - 1]
            )
            nc.vector.tensor_scalar(
                scr_bt2, scr_bt, 1.0, None,
                mybir.AluOpType.mult, mybir.AluOpType.add,
                accum_out=partials[:, 4:5],
            )

            # ship the [128,6] per-partition partials; host reduces.
            nc.sync.dma_start(out=o_d[:, :], in_=partials)

    _split_multi_waits(nc)
    return nc


def _split_multi_waits(nc: bass.Bass):
    """This walrus build accepts only ONE sync-wait per instruction
    ("Too many sync wait commands"). Tile's kernel-tail drain carries one
    wait per live semaphore; peel the extras onto same-engine NOPs that
    each stall on a single semaphore — semantically identical."""
    for bb in nc.main_func.blocks:
        insts = bb.instructions
        i = 0
        while i < len(insts):
            ins = insts[i]
            si = getattr(ins, "sync_info", None)
            if si is not None and si.on_wait is not None and len(si.on_wait) > 1:
                waits = list(si.on_wait)
                nops = []
                for j, w in enumerate(waits[:-1]):
                    nop = mybir.InstNoOp(
                        name=f"{ins.name}-wsplit{j}",
                        sync_info=mybir.SyncInfo(on_wait=[w], on_update=[]),
                        bass_nofuse=True,
                        engine=ins.engine,
                    )
                    nc.register_instruction(nop, overwrite=True)
                    nops.append(nop)
                si.on_wait = [waits[-1]]
                insts[i:i] = nops
                i += len(nops)
            i += 1


_NC = None


def _get_nc() -> bass.Bass:
    global _NC
    if _NC is None:
        _NC = build_bass()
    return _NC


def make_in_maps(probs: np.ndarray, co_matrix: np.ndarray):
    probs = np.ascontiguousarray(np.asarray(probs, dtype=np.float32))
    co_matrix = np.ascontiguousarray(np.asarray(co_matrix, dtype=np.float32))
    in_maps = []
    for k in range(NCORES):
        shift = -SH * k
        p_r = np.ascontiguousarray(
            np.roll(probs, shift, axis=1).astype(ml_dtypes.bfloat16)
        )
        c_r = np.ascontiguousarray(
            np.roll(co_matrix[SH * k : SH * (k + 1), :], shift, axis=1).astype(
                ml_dtypes.bfloat16
            )
        )
        in_maps.append({"probs_r": p_r, "co_r": c_r})
    return in_maps


def finish(outs: np.ndarray) -> np.ndarray:
    """outs: [NCORES, 128, 6] per-partition partials:
    col0/1 = sum_j C*G per bank, col2 = n_pos, col3 = -A/2, col4 = -Bt/2.

    total = A + Bt - 2*CG = -2 * (col3 + col4 + col0 + col1)."""
    o = outs.astype(np.float64)
    total = np.float32(
        -2.0 * (o[:, :, 0] + o[:, :, 1] + o[:, :, 3] + o[:, :, 4]).sum()
    )
    npos = np.float32(o[:, :, 2].sum())
    loss = (total / np.float32(B)) / (npos + np.float32(1e-8))
    return np.array(loss, dtype=np.float32)


TRACE = False
TRACE_DIR = None
LAST_RESULTS = None


def kernel(probs: np.ndarray, co_matrix: np.ndarray) -> np.ndarray:
    global LAST_RESULTS
    nc = _get_nc()
    in_maps = make_in_maps(probs, co_matrix)
    kwargs = {}
    if TRACE:
        kwargs = dict(trace=True, trace_cores=list(range(NCORES)), tmpdir=TRACE_DIR)
    res = run_bass_kernel_spmd(nc, in_maps, list(range(NCORES)), **kwargs)
    LAST_RESULTS = res
    outs = np.stack([r["out"] for r in res.results])
    return finish(outs)


# revision 30
# speedup vs baseline: 1.0867x; 1.0420x over previous
"""Trainium2 Bass kernel for the KB criterion loss.

Math
----
reference:
    diff[b,i,j] = probs[b,j] - probs[b,i]
    loss = sum_ij mean_b (diff^2 * C[i,j]) / (n_pos + 1e-8),  n_pos = count(C > 0)

Expanding the square removes the [B,N,N] intermediate entirely:

    sum_b (P[b,i] - P[b,j])^2 = T_i + T_j - 2*G_ij
        with T_j = sum_b P[b,j]^2   and   G = P^T P  (Gram matrix)

so   total = sum_ij C_ij*T_i + sum_ij C_ij*T_j - 2*sum_ij C_ij*G_ij
           =   A (rows)       +  Bt (cols)      -  2*CG
     loss  = (total / B) / (n_pos + 1e-8)

Sharding (8 cores)
------------------
Shard C by rows: core k owns rows S_k = [128k, 128k+128). P is replicated.
Inputs are column-rolled by 128k so every core runs the same program with
its own row block mapped to local columns [0:128) (so T for the shard rows
is just chunk 0 of the chunked T vector).

The three terms are computed with no cross-engine ladders (v6; each was
measured on HW traces):
  * CG: PE Gram per 512-col PSUM bank (2 bf16 matmuls), DVE multiply by C
    and tensor_scalar-accum row-reduce, pipelined per bank.
  * A = sum_i rowsum_i*T_i: ACT pass over C with the per-partition
    scale AP = (-T_shard/2) straight from PSUM, accum_out per row.
  * Bt = sum_j colsum_j*T_j: both vectors built in PARTITION orientation
    as [128,8] chunk columns via 16 tiny matmuls (contraction over the
    partition dim; chunk k lands in column k), then one tiny DVE
    multiply+reduce. This avoids any [1,N] row-vector op (a 1-partition
    DVE op costs the same as a 128-partition one).
  * n_pos: ACT Sign pass with accum_out.
The per-partition [128,6] partials go straight to HBM; the host does the
final partition/core reduction and the division (the sanctioned scalar
all-reduce).

bf16 inputs (host downcast): halves DMA bytes and avoids multi-pass fp32
matmuls. DMA order P-half0, C, P-half1: the C-gated DVE multiply is the
critical path, not P-half1 (only needed for Gram half 1 / T chunks 4-7).

No PE warmup: the HAM clock gate measurably never opens for this kernel
(bf16 dummy matmuls over 5us of contiguous busy produced zero HAM
transitions), so warmup matmuls only delayed the real work.
"""

import numpy as np
import ml_dtypes

import concourse.bass as bass
import concourse.tile as tile
from concourse import mybir
from concourse.bass_utils import run_bass_kernel_spmd

B = 128
N = 1024
NCORES = 8
SH = N // NCORES  # 128 rows of C per core
NCH = N // SH  # 8 column chunks
F32 = mybir.dt.float32
BF16 = mybir.dt.bfloat16
HALF = 512  # PSUM bank width in fp32


def build_bass() -> bass.Bass:
    nc = bass.Bass()
    p_d = nc.dram_tensor("probs_r", [B, N], BF16, kind="ExternalInput")
    c_d = nc.dram_tensor("co_r", [SH, N], BF16, kind="ExternalInput")
    o_d = nc.dram_tensor("out", [B, 6], F32, kind="ExternalOutput")

    with tile.TileContext(nc) as tc:
        with (
            tc.tile_pool(name="sb", bufs=1) as sb,
            tc.tile_pool(name="ps", bufs=1, space="PSUM") as ps,
        ):
            p_sb = sb.tile([B, N], BF16)
            c_sb = sb.tile([SH, N], BF16)
            psq = sb.tile([B, N], BF16)
            nh_col = sb.tile([B, 1], BF16)
            ones_col = sb.tile([B, 1], BF16)
            scr_mul = sb.tile([SH, N], BF16)
            scr_red = sb.tile([SH, N], BF16)
            scr_cnt = sb.tile([SH, N], BF16)
            scr_a = sb.tile([SH, N], BF16)
            th_sb = sb.tile([B, NCH], F32)
            scr_bt = sb.tile([B, NCH], F32)
            scr_bt2 = sb.tile([B, NCH], F32)
            partials = sb.tile([B, 6], F32)

            d_ps = ps.tile([B, N], F32)  # Gram, banks 0-1
            # th chunk 0 gets its own bank: the ACT A-term pass reads it as
            # a scale AP while the PE is still writing the other chunks —
            # separate banks keep that off the PSUM-collision/serialization
            # path.
            th0_ps = ps.tile([B, 1], F32)
            cols_ps = ps.tile([B, 2 * NCH - 1], F32)  # th 1-7, cs 0-7

            # Constants (DVE-born; matmul operands pair per upstream engine)
            nc.vector.memset(nh_col, -0.5)
            nc.vector.memset(ones_col, 1.0)

            # Loads. P half0 first (heads every chain), then C (gates the
            # critical DVE multiply), then P half1 (only Gram1/th4-7).
            nc.sync.dma_start(out=p_sb[:, 0:HALF], in_=p_d[:, 0:HALF])
            nc.sync.dma_start(out=c_sb, in_=c_d[:, :])
            nc.sync.dma_start(out=p_sb[:, HALF:N], in_=p_d[:, HALF:N])

            # Psq = P*P per half
            nc.vector.tensor_mul(psq[:, 0:HALF], p_sb[:, 0:HALF], p_sb[:, 0:HALF])
            nc.vector.tensor_mul(psq[:, HALF:N], p_sb[:, HALF:N], p_sb[:, HALF:N])

            # PE program. Gram halves head it; the 16 tiny chunk matmuls
            # build th = -T/2 and cs = colsum in partition orientation.
            nc.tensor.matmul(
                d_ps[:, 0:HALF], p_sb[:, 0:SH], p_sb[:, 0:HALF],
                start=True, stop=True,
            )
            nc.tensor.matmul(th0_ps, psq[:, 0:SH], nh_col, start=True, stop=True)
            for k in range(1, 4):
                nc.tensor.matmul(
                    cols_ps[:, k - 1 : k], psq[:, SH * k : SH * (k + 1)], nh_col,
                    start=True, stop=True,
                )
            nc.tensor.matmul(
                d_ps[:, HALF:N], p_sb[:, 0:SH], p_sb[:, HALF:N],
                start=True, stop=True,
            )
            for k in range(4, NCH):
                nc.tensor.matmul(
                    cols_ps[:, k - 1 : k], psq[:, SH * k : SH * (k + 1)], nh_col,
                    start=True, stop=True,
                )
            for k in range(NCH):
                nc.tensor.matmul(
                    cols_ps[:, NCH - 1 + k : NCH + k],
                    c_sb[:, SH * k : SH * (k + 1)], ones_col,
                    start=True, stop=True,
                )

            # DVE: stage -T_shard/2 to SBUF early — the ACT A-term scale AP
            # must be SBUF-resident.
            nc.vector.tensor_copy(th_sb[:, 0:1], th0_ps)

            # DVE: CG per bank (multiply then cheap accum-reduce)
            for h in range(2):
                js = slice(HALF * h, HALF * (h + 1))
                nc.vector.tensor_mul(scr_mul[:, js], c_sb[:, js], d_ps[:, js])
                nc.vector.tensor_scalar(
                    scr_red[:, js], scr_mul[:, js], 1.0, None,
                    mybir.AluOpType.mult, mybir.AluOpType.add,
                    accum_out=partials[:, h : h + 1],
                )

            # ACT: n_pos and the A-term (scale AP = -T_shard/2 from PSUM)
            nc.scalar.activation(
                scr_cnt, c_sb, mybir.ActivationFunctionType.Sign,
                accum_out=partials[:, 2:3],
            )
            nc.scalar.activation(
                scr_a, c_sb, mybir.ActivationFunctionType.Copy,
                scale=th_sb[:, 0:1], accum_out=partials[:, 3:4],
            )

            # DVE: Bt = sum over [128,8] of th * cs
            nc.vector.tensor_copy(th_sb[:, 1:NCH], cols_ps[:, 0 : NCH - 1])
            nc.vector.tensor_mul(
                scr_bt, th_sb, cols_ps[:, NCH - 1 : 2 * NCH - 1]
            )
            nc.vector.tensor_scalar(
                scr_bt2, scr_bt, 1.0, None,
                mybir.AluOpType.mult, mybir.AluOpType.add,
                accum_out=partials[:, 4:5],
            )

            # ship the [128,6] per-partition partials; host reduces.
            nc.sync.dma_start(out=o_d[:, :], in_=partials)

    _split_multi_waits(nc)
    return nc


def _split_multi_waits(nc: bass.Bass):
    """This walrus build accepts only ONE sync-wait per instruction
    ("Too many sync wait commands"). Tile's kernel-tail drain carries one
    wait per live semaphore; peel the extras onto same-engine NOPs that
    each stall on a single semaphore — semantically identical."""
    for bb in nc.main_func.blocks:
        insts = bb.instructions
        i = 0
        while i < len(insts):
            ins = insts[i]
            si = getattr(ins, "sync_info", None)
            if si is not None and si.on_wait is not None and len(si.on_wait) > 1:
                waits = list(si.on_wait)
                nops = []
                for j, w in enumerate(waits[:-1]):
                    nop = mybir.InstNoOp(
                        name=f"{ins.name}-wsplit{j}",
                        sync_info=mybir.SyncInfo(on_wait=[w], on_update=[]),
                        bass_nofuse=True,
                        engine=ins.engine,
                    )
                    nc.register_instruction(nop, overwrite=True)
                    nops.append(nop)
                si.on_wait = [waits[-1]]
                insts[i:i] = nops
                i += len(nops)
            i += 1


_NC = None


def _get_nc() -> bass.Bass:
    global _NC
    if _NC is None:
        _NC = build_bass()
    return _NC


def make_in_maps(probs: np.ndarray, co_matrix: np.ndarray):
    probs = np.ascontiguousarray(np.asarray(probs, dtype=np.float32))
    co_matrix = np.ascontiguousarray(np.asarray(co_matrix, dtype=np.float32))
    in_maps = []
    for k in range(NCORES):
        shift = -SH * k
        p_r = np.ascontiguousarray(
            np.roll(probs, shift, axis=1).astype(ml_dtypes.bfloat16)
        )
        c_r = np.ascontiguousarray(
            np.roll(co_matrix[SH * k : SH * (k + 1), :], shift, axis=1).astype(
                ml_dtypes.bfloat16
            )
        )
        in_maps.append({"probs_r": p_r, "co_r": c_r})
    return in_maps


def finish(outs: np.ndarray) -> np.ndarray:
    """outs: [NCORES, 128, 6] per-partition partials:
    col0/1 = sum_j C*G per bank, col2 = n_pos, col3 = -A/2, col4 = -Bt/2.

    total = A + Bt - 2*CG = -2 * (col3 + col4 + col0 + col1)."""
    o = outs.astype(np.float64)
    total = np.float32(
        -2.0 * (o[:, :, 0] + o[:, :, 1] + o[:, :, 3] + o[:, :, 4]).sum()
    )
    npos = np.float32(o[:, :, 2].sum())
    loss = (total / np.float32(B)) / (npos + np.float32(1e-8))
    return np.array(loss, dtype=np.float32)


TRACE = False
TRACE_DIR = None
LAST_RESULTS = None


def kernel(probs: np.ndarray, co_matrix: np.ndarray) -> np.ndarray:
    global LAST_RESULTS
    nc = _get_nc()
    in_maps = make_in_maps(probs, co_matrix)
    kwargs = {}
    if TRACE:
        kwargs = dict(trace=True, trace_cores=list(range(NCORES)), tmpdir=TRACE_DIR)
    res = run_bass_kernel_spmd(nc, in_maps, list(range(NCORES)), **kwargs)
    LAST_RESULTS = res
    outs = np.stack([r["out"] for r in res.results])
    return finish(outs)


# revision 34
# speedup vs baseline: 1.1251x; 1.0354x over previous
"""Trainium2 Bass kernel for the KB criterion loss.

Math
----
reference:
    diff[b,i,j] = probs[b,j] - probs[b,i]
    loss = sum_ij mean_b (diff^2 * C[i,j]) / (n_pos + 1e-8),  n_pos = count(C > 0)

Expanding the square removes the [B,N,N] intermediate entirely:

    sum_b (P[b,i] - P[b,j])^2 = T_i + T_j - 2*G_ij
        with T_j = sum_b P[b,j]^2   and   G = P^T P  (Gram matrix)

so   total = sum_ij C_ij*T_i + sum_ij C_ij*T_j - 2*sum_ij C_ij*G_ij
           =   A (rows)       +  Bt (cols)      -  2*CG
     loss  = (total / B) / (n_pos + 1e-8)

Sharding (8 cores)
------------------
Shard C by rows: core k owns rows S_k = [128k, 128k+128). P is replicated.
Inputs are column-rolled by 128k so every core runs the same program with
its own row block mapped to local columns [0:128) (so T for the shard rows
is just chunk 0 of the chunked T vector).

The three terms are computed with no cross-engine ladders (v6; each was
measured on HW traces):
  * CG: PE Gram per 512-col PSUM bank (2 bf16 matmuls), DVE multiply by C
    and tensor_scalar-accum row-reduce, pipelined per bank.
  * A = sum_i rowsum_i*T_i: ACT pass over C with the per-partition
    scale AP = (-T_shard/2) straight from PSUM, accum_out per row.
  * Bt = sum_j colsum_j*T_j: both vectors built in PARTITION orientation
    as [128,8] chunk columns via 16 tiny matmuls (contraction over the
    partition dim; chunk k lands in column k), then one tiny DVE
    multiply+reduce. This avoids any [1,N] row-vector op (a 1-partition
    DVE op costs the same as a 128-partition one).
  * n_pos: ACT Sign pass with accum_out.
The per-partition [128,6] partials go straight to HBM; the host does the
final partition/core reduction and the division (the sanctioned scalar
all-reduce).

bf16 inputs (host downcast): halves DMA bytes and avoids multi-pass fp32
matmuls. DMA order P-half0, C, P-half1: the C-gated DVE multiply is the
critical path, not P-half1 (only needed for Gram half 1 / T chunks 4-7).

No PE warmup: the HAM clock gate measurably never opens for this kernel
(bf16 dummy matmuls over 5us of contiguous busy produced zero HAM
transitions), so warmup matmuls only delayed the real work.
"""

import numpy as np
import ml_dtypes

import concourse.bass as bass
import concourse.tile as tile
from concourse import mybir
from concourse.bass_utils import run_bass_kernel_spmd

B = 128
N = 1024
NCORES = 8
SH = N // NCORES  # 128 rows of C per core
NCH = N // SH  # 8 column chunks
F32 = mybir.dt.float32
BF16 = mybir.dt.bfloat16
HALF = 512  # PSUM bank width in fp32


def build_bass() -> bass.Bass:
    nc = bass.Bass()
    p_d = nc.dram_tensor("probs_r", [B, N], BF16, kind="ExternalInput")
    c_d = nc.dram_tensor("co_r", [SH, N], BF16, kind="ExternalInput")
    o_d = nc.dram_tensor("out", [B, 6], F32, kind="ExternalOutput")

    with tile.TileContext(nc) as tc:
        with (
            tc.tile_pool(name="sb", bufs=1) as sb,
            tc.tile_pool(name="ps", bufs=1, space="PSUM") as ps,
        ):
            p_sb = sb.tile([B, N], BF16)
            c_sb = sb.tile([SH, N], BF16)
            # psq as two tiles: Tile tracks dependencies per TILE, so the
            # th chunk matmuls for half 0 must not inherit a wait on the
            # half-1 square.
            psq_a = sb.tile([B, HALF], BF16)
            psq_b = sb.tile([B, HALF], BF16)
            nh_col = sb.tile([B, 1], BF16)
            ones_col = sb.tile([B, 1], BF16)
            scr_mul = sb.tile([SH, N], BF16)
            scr_red = sb.tile([SH, N], BF16)
            scr_cnt = sb.tile([SH, N], BF16)
            scr_a = sb.tile([SH, N], BF16)
            th_sb = sb.tile([B, NCH], F32)
            scr_bt = sb.tile([B, NCH], F32)
            scr_bt2 = sb.tile([B, NCH], F32)
            partials = sb.tile([B, 6], F32)

            # Gram as two per-bank tiles: with one [B,N] tile, the bank-0
            # C*G multiply inherits a wait on the LAST writer of the whole
            # tile — Gram1, which the scheduler places at the very end of
            # the PE queue (measured ~1.5us of dead DVE time).
            d_ps0 = ps.tile([B, HALF], F32)
            d_ps1 = ps.tile([B, HALF], F32)
            # th chunk 0 gets its own bank: the ACT A-term pass reads it as
            # a scale AP while the PE is still writing the other chunks —
            # separate banks keep that off the PSUM-collision/serialization
            # path.
            th0_ps = ps.tile([B, 1], F32)
            cols_ps = ps.tile([B, 2 * NCH - 1], F32)  # th 1-7, cs 0-7

            # Constants (DVE-born; matmul operands pair per upstream engine)
            nc.vector.memset(nh_col, -0.5)
            nc.vector.memset(ones_col, 1.0)

            # Loads. P half0 first (heads every chain), then C (gates the
            # critical DVE multiply), then P half1 (only Gram1/th4-7).
            nc.sync.dma_start(out=p_sb[:, 0:HALF], in_=p_d[:, 0:HALF])
            nc.sync.dma_start(out=c_sb, in_=c_d[:, :])
            nc.sync.dma_start(out=p_sb[:, HALF:N], in_=p_d[:, HALF:N])

            # Psq = P*P per half
            nc.vector.tensor_mul(psq_a, p_sb[:, 0:HALF], p_sb[:, 0:HALF])
            nc.vector.tensor_mul(psq_b, p_sb[:, HALF:N], p_sb[:, HALF:N])

            # PE program. Gram halves head it; the 16 tiny chunk matmuls
            # build th = -T/2 and cs = colsum in partition orientation.
            nc.tensor.matmul(
                d_ps0, p_sb[:, 0:SH], p_sb[:, 0:HALF],
                start=True, stop=True,
            )
            nc.tensor.matmul(th0_ps, psq_a[:, 0:SH], nh_col, start=True, stop=True)
            for k in range(1, 4):
                nc.tensor.matmul(
                    cols_ps[:, k - 1 : k], psq_a[:, SH * k : SH * (k + 1)], nh_col,
                    start=True, stop=True,
                )
            nc.tensor.matmul(
                d_ps1, p_sb[:, 0:SH], p_sb[:, HALF:N],
                start=True, stop=True,
            )
            for k in range(4, NCH):
                nc.tensor.matmul(
                    cols_ps[:, k - 1 : k],
                    psq_b[:, SH * (k - 4) : SH * (k - 3)], nh_col,
                    start=True, stop=True,
                )
            for k in range(NCH):
                nc.tensor.matmul(
                    cols_ps[:, NCH - 1 + k : NCH + k],
                    c_sb[:, SH * k : SH * (k + 1)], ones_col,
                    start=True, stop=True,
                )

            # DVE: stage -T_shard/2 to SBUF early — the ACT A-term scale AP
            # must be SBUF-resident.
            nc.vector.tensor_copy(th_sb[:, 0:1], th0_ps)

            # DVE: CG per bank (multiply then cheap accum-reduce)
            for h, dps in ((0, d_ps0), (1, d_ps1)):
                js = slice(HALF * h, HALF * (h + 1))
                nc.vector.tensor_mul(scr_mul[:, js], c_sb[:, js], dps)
                nc.vector.tensor_scalar(
                    scr_red[:, js], scr_mul[:, js], 1.0, None,
                    mybir.AluOpType.mult, mybir.AluOpType.add,
                    accum_out=partials[:, h : h + 1],
                )

            # ACT: n_pos and the A-term (scale AP = -T_shard/2 from PSUM)
            nc.scalar.activation(
                scr_cnt, c_sb, mybir.ActivationFunctionType.Sign,
                accum_out=partials[:, 2:3],
            )
            nc.scalar.activation(
                scr_a, c_sb, mybir.ActivationFunctionType.Copy,
                scale=th_sb[:, 0:1], accum_out=partials[:, 3:4],
            )

            # DVE: Bt = sum over [128,8] of th * cs
            nc.vector.tensor_copy(th_sb[:, 1:NCH], cols_ps[:, 0 : NCH - 1])
            nc.vector.tensor_mul(
                scr_bt, th_sb, cols_ps[:, NCH - 1 : 2 * NCH - 1]
            )
            nc.vector.tensor_scalar(
                scr_bt2, scr_bt, 1.0, None,
                mybir.AluOpType.mult, mybir.AluOpType.add,
                accum_out=partials[:, 4:5],
            )

            # ship the [128,6] per-partition partials; host reduces.
            nc.sync.dma_start(out=o_d[:, :], in_=partials)

    _split_multi_waits(nc)
    return nc


def _split_multi_waits(nc: bass.Bass):
    """This walrus build accepts only ONE sync-wait per instruction
    ("Too many sync wait commands"). Tile's kernel-tail drain carries one
    wait per live semaphore; peel the extras onto same-engine NOPs that
    each stall on a single semaphore — semantically identical."""
    for bb in nc.main_func.blocks:
        insts = bb.instructions
        i = 0
        while i < len(insts):
            ins = insts[i]
            si = getattr(ins, "sync_info", None)
            if si is not None and si.on_wait is not None and len(si.on_wait) > 1:
                waits = list(si.on_wait)
                nops = []
                for j, w in enumerate(waits[:-1]):
                    nop = mybir.InstNoOp(
                        name=f"{ins.name}-wsplit{j}",
                        sync_info=mybir.SyncInfo(on_wait=[w], on_update=[]),
                        bass_nofuse=True,
                        engine=ins.engine,
                    )
                    nc.register_instruction(nop, overwrite=True)
                    nops.append(nop)
                si.on_wait = [waits[-1]]
                insts[i:i] = nops
                i += len(nops)
            i += 1


_NC = None


def _get_nc() -> bass.Bass:
    global _NC
    if _NC is None:
        _NC = build_bass()
    return _NC


def make_in_maps(probs: np.ndarray, co_matrix: np.ndarray):
    probs = np.ascontiguousarray(np.asarray(probs, dtype=np.float32))
    co_matrix = np.ascontiguousarray(np.asarray(co_matrix, dtype=np.float32))
    in_maps = []
    for k in range(NCORES):
        shift = -SH * k
        p_r = np.ascontiguousarray(
            np.roll(probs, shift, axis=1).astype(ml_dtypes.bfloat16)
        )
        c_r = np.ascontiguousarray(
            np.roll(co_matrix[SH * k : SH * (k + 1), :], shift, axis=1).astype(
                ml_dtypes.bfloat16
            )
        )
        in_maps.append({"probs_r": p_r, "co_r": c_r})
    return in_maps


def finish(outs: np.ndarray) -> np.ndarray:
    """outs: [NCORES, 128, 6] per-partition partials:
    col0/1 = sum_j C*G per bank, col2 = n_pos, col3 = -A/2, col4 = -Bt/2.

    total = A + Bt - 2*CG = -2 * (col3 + col4 + col0 + col1)."""
    o = outs.astype(np.float64)
    total = np.float32(
        -2.0 * (o[:, :, 0] + o[:, :, 1] + o[:, :, 3] + o[:, :, 4]).sum()
    )
    npos = np.float32(o[:, :, 2].sum())
    loss = (total / np.float32(B)) / (npos + np.float32(1e-8))
    return np.array(loss, dtype=np.float32)


TRACE = False
TRACE_DIR = None
LAST_RESULTS = None


def kernel(probs: np.ndarray, co_matrix: np.ndarray) -> np.ndarray:
    global LAST_RESULTS
    nc = _get_nc()
    in_maps = make_in_maps(probs, co_matrix)
    kwargs = {}
    if TRACE:
        kwargs = dict(trace=True, trace_cores=list(range(NCORES)), tmpdir=TRACE_DIR)
    res = run_bass_kernel_spmd(nc, in_maps, list(range(NCORES)), **kwargs)
    LAST_RESULTS = res
    outs = np.stack([r["out"] for r in res.results])
    return finish(outs)
